# revision 8
# baseline (speedup 1.0000x reference)
"""Trainium2 Bass kernel for BioSelfAttention (LIF firing rates + winner-take-all).

Math notes (validated against the jax reference on host):
  * LIF with constant input J and exact reset-to-zero is exactly periodic: the
    spike count over N steps is floor(N / k1) where k1 = ceil(ln(1-1/J)/ln(1-dt/tau))
    (0 spikes if J <= 1 or k1 > N).  ln(1-1/J) is computed as ln(J-1) - ln(J)
    to avoid a division.  rate = cnt * 0.01f matches XLA's cnt/n_steps (it also
    multiplies by the reciprocal).
  * The WTA matrix W = inh*ones + (exc-inh)*I, so x @ W.T = inh*sum(x) + (exc-inh)*x.
    The sum (with the -0.9 factor folded in) is computed on the PE with a
    constant -0.9 weight matrix, which also broadcasts it across partitions.
  * Work is data-parallel over the B*H = 32 (batch, head) pairs: 4 per core.

Layout per core: SBUF tiles are (T=128 partitions, S=4 pairs, D=64) fp32.
"""

import math

import numpy as np

_B, _H, _T, _D = 4, 8, 128, 64
_NCORES = 8
_S = (_B * _H) // _NCORES  # (b,h) pairs per core = 4

_LIF_STEPS = 100
_DECAY = 1.0 - 0.001 / 0.02  # 0.95
_WTA_INH = -0.9
_WTA_EXC = 1.1
_WTA_STEPS = 20

_MAGIC = 8388608.0  # 2^23: (y + MAGIC) - MAGIC == round-to-nearest-even(y)

_nc_cache = {}


def _emit_lif_rate(nc, pool, mybir, J, F, tag):
    """Emit instructions computing LIF firing rates for constant input J.

    J: (128, F) fp32 AP.  Returns a (128, F) fp32 tile AP holding
    rate = floor(100/k1)/100 for J > 1 else 0.
    """
    op = mybir.AluOpType
    act = mybir.ActivationFunctionType
    f32 = mybir.dt.float32

    def t(name):
        return pool.tile([128, F], f32, tag=f"{tag}_{name}", name=f"{tag}_{name}")

    CLN = 1.0 / math.log(_DECAY)  # -19.4957... (f32-rounded at encode)

    # clamp Ln inputs to a tiny positive so J <= 1 lanes stay finite (they are
    # masked to rate 0 at the end); avoids NaN semantics entirely
    tm1 = t("tm1")
    nc.vector.tensor_scalar(tm1, J, 1.0, 1e-30, op.subtract, op.max)  # max(J-1, eps)
    jc = t("jc")
    nc.vector.tensor_scalar(jc, J, 1e-30, None, op.max)
    lt = t("lt")
    nc.scalar.activation(lt, tm1, act.Ln)
    lj = t("lj")
    nc.scalar.activation(lj, jc, act.Ln)
    dd = t("dd")
    nc.vector.tensor_sub(dd, lt, lj)  # ln(1 - 1/J), < 0 for J > 1
    y = t("y")
    # y = dd * CLN, clamped to [0.5, 1000]: lower clamp fixes the degenerate
    # dd == 0 case (huge J where both logs round equal; true k1 is 1 there
    # since any y in (0,1] ceils to 1), upper keeps the MAGIC trick exact.
    nc.vector.tensor_scalar(y, dd, CLN, 0.5, op.mult, op.max)
    nc.vector.tensor_scalar(y, y, 1000.0, None, op.min)
    i0 = t("i0")
    nc.vector.tensor_scalar(i0, y, _MAGIC, _MAGIC, op.add, op.subtract)  # rne(y)
    g = t("g")
    nc.vector.tensor_tensor(g, y, i0, op.is_gt)
    k1 = t("k1")
    nc.vector.tensor_add(k1, i0, g)  # ceil(y), exactly
    r = t("r")
    nc.vector.reciprocal(r, k1)
    p = t("p")
    nc.vector.tensor_scalar(p, r, 100.0, None, op.mult)
    c0m1 = t("c0m1")
    # rne(p) - 1; candidate for floor(100/k1) minus one, fixed up exactly below
    nc.vector.tensor_scalar(c0m1, p, _MAGIC, _MAGIC + 1.0, op.add, op.subtract)
    m1 = t("m1")
    nc.vector.scalar_tensor_tensor(m1, c0m1, 1.0, k1, op.add, op.mult)  # c0*k1
    t1 = t("t1")
    nc.vector.tensor_scalar(t1, m1, 100.0, None, op.is_le)
    m2 = t("m2")
    nc.vector.tensor_add(m2, m1, k1)  # (c0+1)*k1, exact: small ints < 2^24
    t2 = t("t2")
    nc.vector.tensor_scalar(t2, m2, 100.0, None, op.is_le)
    s12 = t("s12")
    nc.vector.tensor_add(s12, t1, t2)
    cc = t("cc")
    nc.vector.tensor_add(cc, c0m1, s12)  # floor(100/k1), exactly
    mask = t("mask")
    nc.vector.tensor_scalar(mask, J, 1.0, None, op.is_gt)
    out = t("out")
    # rate = (floor(100/k1) * 0.01) * (J > 1); all lanes finite by construction
    nc.vector.scalar_tensor_tensor(out, cc, 0.01, mask, op.mult, op.mult)
    return out


def _build_nc():
    import concourse.bacc as bacc
    import concourse.mybir as mybir
    import concourse.tile as tile

    op = mybir.AluOpType
    f32 = mybir.dt.float32
    S, T, D = _S, _T, _D

    nc = bacc.Bacc(
        "TRN2",
        target_bir_lowering=False,
        debug=False,
        enable_asserts=False,
        num_devices=_NCORES,
    )
    qd = nc.dram_tensor("Q", (S, T, D), f32, kind="ExternalInput").ap()
    kd = nc.dram_tensor("K", (S, T, D), f32, kind="ExternalInput").ap()
    vd = nc.dram_tensor("V", (S, T, D), f32, kind="ExternalInput").ap()
    od = nc.dram_tensor("OUT", (S, T, D), f32, kind="ExternalOutput").ap()

    with tile.TileContext(nc) as tc:
        with (
            tc.tile_pool(name="main", bufs=1) as pool,
            tc.tile_pool(name="psum", bufs=2, space="PSUM") as psum_pool,
        ):
            tq = pool.tile([T, S, D], f32)
            tk = pool.tile([T, S, D], f32)
            tv = pool.tile([T, S, D], f32)
            nc.sync.dma_start(tq[:], qd.rearrange("s t d -> t s d"))
            nc.sync.dma_start(tk[:], kd.rearrange("s t d -> t s d"))
            nc.sync.dma_start(tv[:], vd.rearrange("s t d -> t s d"))

            # negative-weight ones matrix: matmul with it computes
            # -0.9 * (sum over partitions) broadcast to all 128 partitions
            negw = pool.tile([128, 128], f32)
            nc.vector.memset(negw[:], _WTA_INH)

            # J1[t, s] = sum_d Q*K
            prod = pool.tile([T, S, D], f32)
            nc.vector.tensor_mul(prod[:], tq[:], tk[:])
            j1 = pool.tile([T, S], f32)
            nc.vector.tensor_reduce(j1[:], prod[:], mybir.AxisListType.X, op.add)

            # stage-1 LIF rates, then 20 WTA iterations on (128, S)
            x1 = _emit_lif_rate(nc, pool, mybir, j1[:], S, "lif1")
            exc = 1.0 + _WTA_EXC - _WTA_INH  # x + x@W.T = 3*x - 0.9*S
            for _ in range(_WTA_STEPS):
                ps1 = psum_pool.tile([T, S], f32, tag="ps1")
                nc.tensor.matmul(ps1[:], negw[:], x1)
                y1 = pool.tile([T, S], f32, tag="y1")
                nc.vector.scalar_tensor_tensor(y1[:], x1, exc, ps1[:], op.mult, op.add)
                nc.vector.tensor_scalar(x1, y1[:], 0.0, 1.0, op.max, op.min)

            # J2[t, s, d] = rates1[t, s] * V[t, s, d]
            jv = pool.tile([T, S, D], f32)
            for s in range(S):
                nc.vector.tensor_scalar(
                    jv[:, s, :], tv[:, s, :], x1[:, s : s + 1], None, op.mult
                )

            # stage-2 LIF rates, then 20 WTA iterations on (128, S*D)
            x2 = _emit_lif_rate(nc, pool, mybir, jv[:], S * D, "lif2")
            x2v = x2.rearrange("t (s d) -> t s d", d=D)
            for _ in range(_WTA_STEPS):
                ps2 = psum_pool.tile([T, S, D], f32, tag="ps2")
                nc.tensor.matmul(ps2[:], negw[:], x2v)
                ns = pool.tile([T, S], f32, tag="ns")
                nc.vector.tensor_reduce(ns[:], ps2[:], mybir.AxisListType.X, op.add)
                y2 = pool.tile([T, S, D], f32, tag="y2")
                for s in range(S):
                    nc.vector.tensor_scalar(
                        y2[:, s, :], x2v[:, s, :], exc, ns[:, s : s + 1], op.mult, op.add
                    )
                nc.vector.tensor_scalar(x2v, y2[:], 0.0, 1.0, op.max, op.min)

            nc.sync.dma_start(od.rearrange("s t d -> t s d"), x2v)

    nc.compile()
    return nc


def _get_nc():
    if "nc" not in _nc_cache:
        _nc_cache["nc"] = _build_nc()
    return _nc_cache["nc"]


def run(Q, K, V, **spmd_kwargs):
    from concourse.bass_utils import run_bass_kernel_spmd

    nc = _get_nc()
    Qr = np.ascontiguousarray(Q, dtype=np.float32).reshape(_NCORES, _S, _T, _D)
    Kr = np.ascontiguousarray(K, dtype=np.float32).reshape(_NCORES, _S, _T, _D)
    Vr = np.ascontiguousarray(V, dtype=np.float32).reshape(_NCORES, _S, _T, _D)
    in_maps = [{"Q": Qr[c], "K": Kr[c], "V": Vr[c]} for c in range(_NCORES)]
    return run_bass_kernel_spmd(nc, in_maps, core_ids=list(range(_NCORES)), **spmd_kwargs)


def kernel(Q, K, V):
    res = run(Q, K, V)
    out = np.stack([res.results[c]["OUT"] for c in range(_NCORES)])
    return out.reshape(_B, _H, _T, _D)


# revision 11
# speedup vs baseline: 1.5994x; 1.5994x over previous
"""Trainium2 Bass kernel for BioSelfAttention (LIF firing rates + winner-take-all).

Math notes (validated against the jax reference on host):
  * LIF with constant input J and exact reset-to-zero is exactly periodic: the
    spike count over N=100 steps is floor(N / k1) with
    k1 = ceil(ln(1-1/J)/ln(0.95)) (0 spikes if J <= 1 or k1 > N).
    ln(1-1/J) = ln(J-1) - ln(J) avoids a division; floor/ceil are computed
    exactly in f32 with the 2^23 round-to-nearest trick plus a compare, and
    floor(100/k1) via an approximate reciprocal candidate fixed up with one
    exact integer comparison (all products < 2^24 are exact in f32).
  * The WTA matrix W = inh*ones + (exc-inh)*I, so x @ W.T = inh*sum(x) + 2*x.
    The per-pair sum is computed on the PE with a constant -0.9 ones matrix
    (bf16), which also broadcasts it across partitions.  The WTA state lives
    in bf16; each iteration is ONE fused custom-DVE op per pair:
    x <- clip(3x + nS, 0, 1) with the row-sums for the next iteration coming
    out of the same instruction's accumulator (WTA2).
  * Work is data-parallel over the B*H = 32 (batch, head) pairs: 4 per core.

Layout per core: SBUF tiles are (T=128 partitions, S=4 pairs, D=64).
"""

import math

import numpy as np

_B, _H, _T, _D = 4, 8, 128, 64
_NCORES = 8
_S = (_B * _H) // _NCORES  # (b,h) pairs per core = 4

_DECAY = 1.0 - 0.001 / 0.02  # 0.95
_WTA_INH = -0.9
_WTA_STEPS = 20

_MAGIC = 8388608.0  # 2^23: (y + MAGIC) - MAGIC == round-to-nearest-even(y)
_EPS = 1e-30
_CLN = 1.0 / math.log(_DECAY)

_cache = {}


def _f32(x):
    return np.asarray(x, np.float32) if isinstance(x, np.ndarray) else np.float32(x)


def _register_dve_ops():
    """Append the fused ops this kernel uses to the custom-DVE registry."""
    import concourse.dve_ops as D
    from concourse.dve_spec import (
        Spec, Src0, Src1, C0, C1, C2, Zero, One, maxx, minn, lower,
    )
    from concourse.dve_spec import _has_src1 as has_src1
    from concourse.dve_uop import DveOpSpec, AluOp

    if "BIO_WTA_STEP_T" in D._SUB_OPCODE_FOR_NAME:
        return D

    def add_op(name, spec, subdim=False):
        row = D._CUSTOM_DVE_ROW_BASE + len(D.OPS)
        assert row < 0x20
        D._SUB_OPCODE_FOR_NAME[name] = row
        shas = {}
        for ver in ("v3", "v4"):
            try:
                res = DveOpSpec(
                    name=name, opcode=row, uops=lower(spec, ver=ver),
                    rd1_en=has_src1(spec),
                )
                shas[ver] = res.sha(ver)
            except Exception:
                pass
        op = D.DveOp(name, spec, subdim, shas)
        D.OPS.append(op)
        D.CUSTOM_DVE_SPECS[name] = spec
        return op

    F = _f32

    # x <- clip(x*s0 + nS, 0, 1); nS arrives as a same-shape stream (in1)
    add_op("BIO_WTA_STEP_T", Spec(
        body=minn(maxx(Src0 * C0 + Src1, Zero), One),
        reference=lambda in0, in1, s0, s1, imm2: np.clip(
            F(F(F(in0) * F(s0)) + F(in1)), 0.0, 1.0),
    ))
    # x <- clip(x*s0 + nS[p], 0, 1), accum_out = row-sum of the clipped x
    add_op("BIO_WTA_STEP_A", Spec(
        body=minn(maxx(Src0 * C0 + C1, Zero), One),
        accum=AluOp.ADD,
        reference=lambda in0, in1, s0, s1, imm2: (lambda o: (o, o.sum(-1, keepdims=True, dtype=np.float32)))(
            np.clip(F(F(F(in0) * F(s0)) + F(s1)), 0.0, 1.0)),
    ))
    # y = clamp((lt - lj)*C, 0.5, 1000)
    add_op("BIO_LIF_Y", Spec(
        body=minn(maxx((Src0 - Src1) * C0, C1), C2),
        reference=lambda in0, in1, s0, s1, imm2: np.minimum(
            np.maximum(F(F(F(in0) - F(in1)) * F(s0)), F(s1)), F(imm2)),
    ))
    # k1 = ceil(y) exactly: i0 = rne(y) via magic add/sub, then +[y > i0]
    def _ceil_ref(in0, in1, s0, s1, imm2):
        i0 = F(F(F(in0) + F(s0)) - F(s0))
        return F(i0 + F(F(in0) > i0))
    add_op("BIO_LIF_CEIL", Spec(
        body=(lambda i0: i0 + (Src0 > i0))((Src0 + C0) - C0),
        reference=_ceil_ref,
    ))
    # cc = floor(100/k1) exactly from approximate r ~ 1/k1 (in0) and k1 (in1):
    # c0m1 = rne(100 r) - 1;  cc = c0m1 + [ (c0m1+1)*k1 <= 100 ]
    def _cnt_ref(in0, in1, s0, s1, imm2):
        p = F(F(in0) * F(s0))
        c0m1 = F(F(p + F(s1)) - F(imm2))
        m1 = F(F(c0m1 + np.float32(1.0)) * F(in1))
        return F(c0m1 + F(m1 <= F(s0)))
    def _cnt_body():
        p = Src0 * C0
        c0m1 = (p + C1) - C2
        m1 = (c0m1 + One) * Src1
        return c0m1 + (m1 <= C0)
    add_op("BIO_LIF_CNT", Spec(body=_cnt_body(), reference=_cnt_ref))
    # rate = (cc*s0) * [J > s1]
    add_op("BIO_LIF_RATE", Spec(
        body=(Src0 * C0) * (Src1 > C1),
        reference=lambda in0, in1, s0, s1, imm2: F(
            F(F(in0) * F(s0)) * F(F(in1) > F(s1))),
    ))
    return D


def _emit_lif_rate(nc, pool, mybir, dve, J, F, tag):
    """LIF firing rates for constant input J: (128, F) f32 -> (128, F) f32."""
    op = mybir.AluOpType
    act = mybir.ActivationFunctionType
    f32 = mybir.dt.float32

    def t(name):
        return pool.tile([128, F], f32, tag=f"{tag}_{name}", name=f"{tag}_{name}")

    tm1 = t("tm1")
    nc.vector.tensor_scalar(tm1, J, 1.0, _EPS, op.subtract, op.max)
    jc = t("jc")
    nc.vector.tensor_scalar(jc, J, _EPS, None, op.max)
    lt = t("lt")
    nc.scalar.activation(lt, tm1, act.Ln)
    lj = t("lj")
    nc.scalar.activation(lj, jc, act.Ln)
    y = t("y")
    nc.vector._custom_dve(dve["BIO_LIF_Y"], out=y, in0=lt, in1=lj,
                          s0=_CLN, s1=0.5, imm2=1000.0)
    k1 = t("k1")
    nc.vector._custom_dve(dve["BIO_LIF_CEIL"], out=k1, in0=y, s0=_MAGIC)
    r = t("r")
    nc.vector.reciprocal_approx_fast(out=r, in_=k1)
    cc = t("cc")
    nc.vector._custom_dve(dve["BIO_LIF_CNT"], out=cc, in0=r, in1=k1,
                          s0=100.0, s1=_MAGIC, imm2=_MAGIC + 1.0)
    out = t("out")
    nc.vector._custom_dve(dve["BIO_LIF_RATE"], out=out, in0=cc, in1=J,
                          s0=0.01, s1=1.0)
    return out


def _build_nc():
    import concourse.bacc as bacc
    import concourse.mybir as mybir
    import concourse.tile as tile

    D_ops = _register_dve_ops()
    dve = {o.name: o for o in D_ops.OPS}

    op = mybir.AluOpType
    act = mybir.ActivationFunctionType
    f32 = mybir.dt.float32
    bf16 = mybir.dt.bfloat16
    S, T, D = _S, _T, _D

    nc = bacc.Bacc(
        "TRN2",
        target_bir_lowering=False,
        debug=False,
        enable_asserts=False,
        num_devices=_NCORES,
    )
    qd = nc.dram_tensor("Q", (S, T, D), f32, kind="ExternalInput").ap()
    kd = nc.dram_tensor("K", (S, T, D), f32, kind="ExternalInput").ap()
    vd = nc.dram_tensor("V", (S, T, D), f32, kind="ExternalInput").ap()
    od = nc.dram_tensor("OUT", (S, T, D), f32, kind="ExternalOutput").ap()

    with tile.TileContext(nc) as tc:
        with (
            tc.tile_pool(name="main", bufs=1) as pool,
            tc.tile_pool(name="psum", bufs=2, space="PSUM") as psum_pool,
        ):
            # dummy Ln up front so the ACT table load overlaps the DMAs
            warm = pool.tile([128, 1], f32)
            nc.vector.memset(warm, 1.0)
            nc.scalar.activation(warm, warm, act.Ln)

            tq = pool.tile([T, S, D], f32)
            tk = pool.tile([T, S, D], f32)
            tv = pool.tile([T, S, D], f32)
            nc.sync.dma_start(tq[:], qd.rearrange("s t d -> t s d"))
            nc.sync.dma_start(tk[:], kd.rearrange("s t d -> t s d"))
            nc.sync.dma_start(tv[:], vd.rearrange("s t d -> t s d"))

            # -0.9 ones matrix (bf16): matmul computes -0.9 * colsum broadcast
            negw = pool.tile([128, 128], bf16)
            nc.vector.memset(negw[:], _WTA_INH)

            # J1[t, s] = sum_d Q*K
            prod = pool.tile([T, S, D], f32)
            nc.vector.tensor_mul(prod[:], tq[:], tk[:])
            j1 = pool.tile([T, S], f32)
            nc.vector.tensor_reduce(j1[:], prod[:], mybir.AxisListType.X, op.add)

            # stage-1 LIF rates -> 20 WTA iterations on (128, S), bf16 state
            rate1 = _emit_lif_rate(nc, pool, mybir, dve, j1[:], S, "lif1")
            x1b = pool.tile([T, S], bf16)
            nc.vector.tensor_copy(x1b[:], rate1)
            for _ in range(_WTA_STEPS):
                ps1 = psum_pool.tile([T, S], f32, tag="ps1")
                nc.tensor.matmul(ps1[:], negw[:], x1b[:])
                nc.vector._custom_dve(dve["BIO_WTA_STEP_T"], out=x1b[:],
                                      in0=x1b[:], in1=ps1[:], s0=3.0)
            x1f = pool.tile([T, S], f32)
            nc.vector.tensor_copy(x1f[:], x1b[:])

            # J2[t, s, d] = rates1[t, s] * V[t, s, d]
            jv = pool.tile([T, S, D], f32)
            for s in range(S):
                nc.vector.tensor_scalar(
                    jv[:, s, :], tv[:, s, :], x1f[:, s : s + 1], None, op.mult
                )

            # stage-2 LIF rates -> 20 WTA iterations on (128, S*D), bf16 state
            rate2 = _emit_lif_rate(nc, pool, mybir, dve, jv[:], S * D, "lif2")
            x2b = pool.tile([T, S, D], bf16)
            nc.vector.tensor_copy(x2b[:], rate2)
            rs = pool.tile([T, S], f32)
            nc.vector.tensor_reduce(rs[:], x2b[:], mybir.AxisListType.X, op.add)
            for _ in range(_WTA_STEPS):
                rsb = pool.tile([T, S], bf16, tag="rsb")
                nc.vector.tensor_copy(rsb[:], rs[:])
                ps2 = psum_pool.tile([T, S], f32, tag="ps2")
                nc.tensor.matmul(ps2[:], negw[:], rsb[:])
                for s in range(S):
                    nc.vector._custom_dve(
                        dve["BIO_WTA_STEP_A"], out=x2b[:, s, :], in0=x2b[:, s, :],
                        s0=3.0, s1=ps2[:, s : s + 1],
                        accum_out=rs[:, s : s + 1],
                    )
            x2f = pool.tile([T, S, D], f32)
            nc.vector.tensor_copy(x2f[:], x2b[:])

            nc.sync.dma_start(od.rearrange("s t d -> t s d"), x2f[:])

    nc.compile()
    return nc


def _get_nc():
    if "nc" not in _cache:
        _cache["nc"] = _build_nc()
    return _cache["nc"]


def run(Q, K, V, **spmd_kwargs):
    from concourse.bass_utils import run_bass_kernel_spmd

    nc = _get_nc()
    Qr = np.ascontiguousarray(Q, dtype=np.float32).reshape(_NCORES, _S, _T, _D)
    Kr = np.ascontiguousarray(K, dtype=np.float32).reshape(_NCORES, _S, _T, _D)
    Vr = np.ascontiguousarray(V, dtype=np.float32).reshape(_NCORES, _S, _T, _D)
    in_maps = [{"Q": Qr[c], "K": Kr[c], "V": Vr[c]} for c in range(_NCORES)]
    return run_bass_kernel_spmd(nc, in_maps, core_ids=list(range(_NCORES)), **spmd_kwargs)


def kernel(Q, K, V):
    res = run(Q, K, V)
    out = np.stack([res.results[c]["OUT"] for c in range(_NCORES)])
    return out.reshape(_B, _H, _T, _D)


# revision 12
# speedup vs baseline: 1.9682x; 1.2306x over previous
"""Trainium2 Bass kernel for BioSelfAttention (LIF firing rates + winner-take-all).

Math notes (validated against the jax reference on host):
  * LIF with constant input J and exact reset-to-zero is exactly periodic: the
    spike count over N=100 steps is floor(N / k1) with
    k1 = ceil(ln(1-1/J)/ln(0.95)) (0 spikes if J <= 1 or k1 > N).
    ln(1-1/J) = ln(J-1) - ln(J) avoids a division; floor/ceil are computed
    exactly in f32 with the 2^23 round-to-nearest trick plus a compare, and
    floor(100/k1) via an approximate reciprocal candidate fixed up with one
    exact integer comparison (all products < 2^24 are exact in f32).
  * The WTA matrix W = inh*ones + (exc-inh)*I, so x @ W.T = inh*sum(x) + 2*x.
    The per-pair sum is computed on the PE with a constant -0.9 ones matrix
    (bf16), which also broadcasts it across partitions.  The WTA state lives
    in bf16; each iteration is ONE fused custom-DVE op per pair:
    x <- clip(3x + nS, 0, 1) with the row-sums for the next iteration coming
    out of the same instruction's accumulator (WTA2).
  * Work is data-parallel over the B*H = 32 (batch, head) pairs: 4 per core.

Layout per core: SBUF tiles are (T=128 partitions, S=4 pairs, D=64).
"""

import math

import numpy as np

_B, _H, _T, _D = 4, 8, 128, 64
_NCORES = 8
_S = (_B * _H) // _NCORES  # (b,h) pairs per core = 4

_DECAY = 1.0 - 0.001 / 0.02  # 0.95
_WTA_INH = -0.9
_WTA_STEPS = 20

_MAGIC = 8388608.0  # 2^23: (y + MAGIC) - MAGIC == round-to-nearest-even(y)
_EPS = 1e-30
_CLN = 1.0 / math.log(_DECAY)

_cache = {}


def _f32(x):
    return np.asarray(x, np.float32) if isinstance(x, np.ndarray) else np.float32(x)


def _register_dve_ops():
    """Append the fused ops this kernel uses to the custom-DVE registry."""
    import concourse.dve_ops as D
    from concourse.dve_spec import (
        Spec, Src0, Src1, C0, C1, C2, Zero, One, maxx, minn, lower,
    )
    from concourse.dve_spec import _has_src1 as has_src1
    from concourse.dve_uop import DveOpSpec, AluOp

    if "BIO_WTA_STEP_T" in D._SUB_OPCODE_FOR_NAME:
        return D

    def add_op(name, spec, subdim=False):
        row = D._CUSTOM_DVE_ROW_BASE + len(D.OPS)
        assert row < 0x20
        D._SUB_OPCODE_FOR_NAME[name] = row
        shas = {}
        for ver in ("v3", "v4"):
            try:
                res = DveOpSpec(
                    name=name, opcode=row, uops=lower(spec, ver=ver),
                    rd1_en=has_src1(spec),
                )
                shas[ver] = res.sha(ver)
            except Exception:
                pass
        op = D.DveOp(name, spec, subdim, shas)
        D.OPS.append(op)
        D.CUSTOM_DVE_SPECS[name] = spec
        return op

    F = _f32

    # x <- clip(x*s0 + nS, 0, 1); nS arrives as a same-shape stream (in1)
    add_op("BIO_WTA_STEP_T", Spec(
        body=minn(maxx(Src0 * C0 + Src1, Zero), One),
        reference=lambda in0, in1, s0, s1, imm2: np.clip(
            F(F(F(in0) * F(s0)) + F(in1)), 0.0, 1.0),
    ))
    # x <- clip(x*s0 + nS[p], 0, 1), accum_out = row-sum of the clipped x
    add_op("BIO_WTA_STEP_A", Spec(
        body=minn(maxx(Src0 * C0 + C1, Zero), One),
        accum=AluOp.ADD,
        reference=lambda in0, in1, s0, s1, imm2: (lambda o: (o, o.sum(-1, keepdims=True, dtype=np.float32)))(
            np.clip(F(F(F(in0) * F(s0)) + F(s1)), 0.0, 1.0)),
    ))
    # y = clamp((lt - lj)*C, 0.5, 1000)
    add_op("BIO_LIF_Y", Spec(
        body=minn(maxx((Src0 - Src1) * C0, C1), C2),
        reference=lambda in0, in1, s0, s1, imm2: np.minimum(
            np.maximum(F(F(F(in0) - F(in1)) * F(s0)), F(s1)), F(imm2)),
    ))
    # k1 = ceil(y) exactly: i0 = rne(y) via magic add/sub, then +[y > i0]
    def _ceil_ref(in0, in1, s0, s1, imm2):
        i0 = F(F(F(in0) + F(s0)) - F(s0))
        return F(i0 + F(F(in0) > i0))
    add_op("BIO_LIF_CEIL", Spec(
        body=(lambda i0: i0 + (Src0 > i0))((Src0 + C0) - C0),
        reference=_ceil_ref,
    ))
    # cc = floor(100/k1) exactly from approximate r ~ 1/k1 (in0) and k1 (in1):
    # c0m1 = rne(100 r) - 1;  cc = c0m1 + [ (c0m1+1)*k1 <= 100 ]
    def _cnt_ref(in0, in1, s0, s1, imm2):
        p = F(F(in0) * F(s0))
        c0m1 = F(F(p + F(s1)) - F(imm2))
        m1 = F(F(c0m1 + np.float32(1.0)) * F(in1))
        return F(c0m1 + F(m1 <= F(s0)))
    def _cnt_body():
        p = Src0 * C0
        c0m1 = (p + C1) - C2
        m1 = (c0m1 + One) * Src1
        return c0m1 + (m1 <= C0)
    add_op("BIO_LIF_CNT", Spec(body=_cnt_body(), reference=_cnt_ref))
    # rate = (cc*s0) * [J > s1]
    add_op("BIO_LIF_RATE", Spec(
        body=(Src0 * C0) * (Src1 > C1),
        reference=lambda in0, in1, s0, s1, imm2: F(
            F(F(in0) * F(s0)) * F(F(in1) > F(s1))),
    ))
    return D


def _emit_lif_rate(nc, pool, mybir, dve, J, F, tag):
    """LIF firing rates for constant input J: (128, F) f32 -> (128, F) f32."""
    op = mybir.AluOpType
    act = mybir.ActivationFunctionType
    f32 = mybir.dt.float32

    def t(name):
        return pool.tile([128, F], f32, tag=f"{tag}_{name}", name=f"{tag}_{name}")

    tm1 = t("tm1")
    nc.vector.tensor_scalar(tm1, J, 1.0, _EPS, op.subtract, op.max)
    jc = t("jc")
    nc.vector.tensor_scalar(jc, J, _EPS, None, op.max)
    lt = t("lt")
    nc.scalar.activation(lt, tm1, act.Ln)
    lj = t("lj")
    nc.scalar.activation(lj, jc, act.Ln)
    y = t("y")
    nc.vector._custom_dve(dve["BIO_LIF_Y"], out=y, in0=lt, in1=lj,
                          s0=_CLN, s1=0.5, imm2=1000.0)
    k1 = t("k1")
    nc.vector._custom_dve(dve["BIO_LIF_CEIL"], out=k1, in0=y, s0=_MAGIC)
    r = t("r")
    nc.vector.reciprocal_approx_fast(out=r, in_=k1)
    cc = t("cc")
    nc.vector._custom_dve(dve["BIO_LIF_CNT"], out=cc, in0=r, in1=k1,
                          s0=100.0, s1=_MAGIC, imm2=_MAGIC + 1.0)
    out = t("out")
    nc.vector._custom_dve(dve["BIO_LIF_RATE"], out=out, in0=cc, in1=J,
                          s0=0.01, s1=1.0)
    return out


def _build_nc():
    import concourse.bacc as bacc
    import concourse.mybir as mybir
    import concourse.tile as tile

    D_ops = _register_dve_ops()
    dve = {o.name: o for o in D_ops.OPS}

    op = mybir.AluOpType
    act = mybir.ActivationFunctionType
    f32 = mybir.dt.float32
    bf16 = mybir.dt.bfloat16
    S, T, D = _S, _T, _D

    nc = bacc.Bacc(
        "TRN2",
        target_bir_lowering=False,
        debug=False,
        enable_asserts=False,
        num_devices=_NCORES,
    )
    qd = nc.dram_tensor("Q", (S, T, D), f32, kind="ExternalInput").ap()
    kd = nc.dram_tensor("K", (S, T, D), f32, kind="ExternalInput").ap()
    vd = nc.dram_tensor("V", (S, T, D), f32, kind="ExternalInput").ap()
    od = nc.dram_tensor("OUT", (S, T, D), f32, kind="ExternalOutput").ap()

    # Packed layout: partition p = 32*s + (t >> 2), free = (t & 3, d).
    # Every partition holds elements of exactly one (b,h) pair, so per-pair
    # WTA sums are per-partition row sums (fused-op accumulators) reduced
    # across each 32-partition group by one tiny block-diagonal matmul.
    A_, B_ = 32, 4  # t = 4*a + b

    def packed(ap):
        return ap.rearrange("s (a b) d -> (s a) b d", a=A_, b=B_)

    def wta_loop(pool, psum_pool, x, acc, F, tag):
        """20 iterations of x <- clip(3x - 0.9*S_pair, 0, 1) on packed x."""
        nonlocal mb
        for _ in range(_WTA_STEPS):
            accb = pool.tile([T, 1], bf16, tag=f"{tag}_accb", name=f"{tag}_accb")
            nc.vector.tensor_copy(accb[:], acc)
            ns = psum_pool.tile([T, 1], f32, tag=f"{tag}_ns")
            nc.tensor.matmul(ns[:], mb[:], accb[:])
            nc.vector._custom_dve(dve["BIO_WTA_STEP_A"], out=x, in0=x,
                                  s0=3.0, s1=ns[:], accum_out=acc)

    with tile.TileContext(nc) as tc:
        with (
            tc.tile_pool(name="main", bufs=1) as pool,
            tc.tile_pool(name="psum", bufs=2, space="PSUM") as psum_pool,
        ):
            # dummy Ln up front so the ACT table load overlaps the DMAs
            warm = pool.tile([128, 1], f32)
            nc.vector.memset(warm, 1.0)
            nc.scalar.activation(warm, warm, act.Ln)

            tq = pool.tile([T, B_, D], f32)
            tk = pool.tile([T, B_, D], f32)
            tv = pool.tile([T, B_, D], f32)
            nc.sync.dma_start(tq[:], packed(qd))
            nc.sync.dma_start(tk[:], packed(kd))
            nc.sync.dma_start(tv[:], packed(vd))

            # block-diagonal -0.9 matrix (bf16): matmul of the per-partition
            # row sums against it yields -0.9 * (pair sum) on every partition
            mb = pool.tile([128, 128], bf16)
            nc.gpsimd.memset(mb[:], 0.0)
            for s in range(S):
                nc.gpsimd.memset(mb[32 * s : 32 * (s + 1), 32 * s : 32 * (s + 1)],
                                 _WTA_INH)

            # J1[p, b] = sum_d Q*K
            prod = pool.tile([T, B_, D], f32)
            nc.vector.tensor_mul(prod[:], tq[:], tk[:])
            j1 = pool.tile([T, B_], f32)
            nc.vector.tensor_reduce(j1[:], prod[:], mybir.AxisListType.X, op.add)

            # stage-1 LIF rates -> 20 WTA iterations on (128, 4)
            x1 = _emit_lif_rate(nc, pool, mybir, dve, j1[:], B_, "lif1")
            acc1 = pool.tile([T, 1], f32)
            nc.vector.tensor_reduce(acc1[:], x1, mybir.AxisListType.X, op.add)
            wta_loop(pool, psum_pool, x1, acc1[:], B_, "w1")

            # J2[p, b, d] = rates1[p, b] * V[p, b, d]
            jv = pool.tile([T, B_, D], f32)
            for b in range(B_):
                nc.vector.tensor_scalar(
                    jv[:, b, :], tv[:, b, :], x1[:, b : b + 1], None, op.mult
                )

            # stage-2 LIF rates -> 20 WTA iterations on (128, 256)
            rate2 = _emit_lif_rate(nc, pool, mybir, dve, jv[:], B_ * D, "lif2")
            x2 = rate2.rearrange("t (b d) -> t b d", d=D)
            acc2 = pool.tile([T, 1], f32)
            nc.vector.tensor_reduce(acc2[:], x2, mybir.AxisListType.XY, op.add)
            wta_loop(pool, psum_pool, x2, acc2[:], B_ * D, "w2")

            nc.sync.dma_start(packed(od), x2)

    nc.compile()
    return nc


def _get_nc():
    if "nc" not in _cache:
        _cache["nc"] = _build_nc()
    return _cache["nc"]


def run(Q, K, V, **spmd_kwargs):
    from concourse.bass_utils import run_bass_kernel_spmd

    nc = _get_nc()
    Qr = np.ascontiguousarray(Q, dtype=np.float32).reshape(_NCORES, _S, _T, _D)
    Kr = np.ascontiguousarray(K, dtype=np.float32).reshape(_NCORES, _S, _T, _D)
    Vr = np.ascontiguousarray(V, dtype=np.float32).reshape(_NCORES, _S, _T, _D)
    in_maps = [{"Q": Qr[c], "K": Kr[c], "V": Vr[c]} for c in range(_NCORES)]
    return run_bass_kernel_spmd(nc, in_maps, core_ids=list(range(_NCORES)), **spmd_kwargs)


def kernel(Q, K, V):
    res = run(Q, K, V)
    out = np.stack([res.results[c]["OUT"] for c in range(_NCORES)])
    return out.reshape(_B, _H, _T, _D)
